# revision 4
# baseline (speedup 1.0000x reference)
"""Trainium2 Bass kernel: gate-merged tensor-train (TT) MoE layer.

Reference math (per batch element b):
    merge each TT core over experts with gates[b]  ->  C_i  (tiny, <=512 floats)
    then an 8-step TT contraction maps X[b] : [512, 4096] -> Y[b] : [512, 4096].

Because every boundary rank is 8, the whole chain collapses exactly to a
rank-8 factorization:
    Y_b = X_b @ W4_b @ E4_b
with W4_b = C0*C1*C2*C3 reshaped to [4096, 8] and E4_b = C4*C5*C6*C7
reshaped to [8, 4096].  The factor merge is ~0.5 MFLOP of 8x8-sized einsums
(done on host at float64); all heavy lifting (16 MiB of streaming, the
4096-wide contractions) runs on the NeuronCores.

Sharding: data-parallel over batch B=8 across the 8 cores (one batch
element per core); the merged factors are per-core constants.

On-device dataflow per core:
    X  --DMA-->  SBUF (4 s-chunks of [128, 4096])
    PE transpose (fp32, identity matmul) -> PSUM -> ACT copy -> XT (feat-major)
    mm1 (f32r):  t4[8, s] += W4_chunk^T @ XT_chunk  (32 K=128 chunks, PSUM acc)
    mm2 (f32r):  Y[s-chunk, n-chunk] = t4_chunk^T @ E4  -> PSUM -> DVE -> SBUF
    DMA out, row-contiguous.
"""

import numpy as np

B = 8
S = 512
F = 4096
R = 8
N_CORES = 8

_PROGRAM = None
LAST_RESULTS = None  # BassKernelResults of the most recent run (for profiling)
TRACE = False        # set True (e.g. from test.py) to capture an NTFF profile


def _merge_factors(gates, core_first, cores_mid, core_last):
    """Host-side merge of the tiny TT cores -> per-batch rank-8 factors.

    Returns (W4, E4): [B, 4096, 8] and [B, 8, 4096] float32.
    """
    g = np.asarray(gates, np.float64)
    cf = np.asarray(core_first, np.float64)   # [E, 1, 8, 8]
    cm = np.asarray(cores_mid, np.float64)    # [6, E, 8, 8, 8]
    cl = np.asarray(core_last, np.float64)    # [E, 8, 8, 1]

    W4 = np.empty((B, F, R), np.float32)
    E4 = np.empty((B, R, F), np.float32)
    for b in range(B):
        C = [np.einsum('e,ermp->rmp', g[b], cf)] + \
            [np.einsum('e,ermp->rmp', g[b], cm[i]) for i in range(6)] + \
            [np.einsum('e,ermp->rmp', g[b], cl)]
        C0 = C[0][0]                                  # [m0, p1]
        W = np.einsum('ab,bic->iac', C0, C[1])        # [m1, m0, p2]
        W = np.einsum('iac,cjd->jiad', W, C[2])       # [m2, m1, m0, p3]
        W = np.einsum('jiad,dke->kjiae', W, C[3])     # [m3, m2, m1, m0, p4]
        W4[b] = W.reshape(F, R)
        E = np.einsum('anb,bmc->anmc', C[4], C[5])    # [p4, n0, n1, p6]
        E = np.einsum('anmc,ckd->anmkd', E, C[6])     # [p4, n0, n1, n2, p7]
        E = np.einsum('anmkd,dl->anmkl', E, C[7][:, :, 0])
        E4[b] = E.reshape(R, F)
    return W4, E4


def _build_program():
    """Build + compile the per-core Bass/Tile program (identical on all cores)."""
    from contextlib import ExitStack
    import concourse.bass as bass
    import concourse.tile as tile
    from concourse import bacc, mybir

    f32 = mybir.dt.float32
    f32r = mybir.dt.float32r

    nc = bacc.Bacc("TRN2", target_bir_lowering=False, debug=False)
    x_d = nc.dram_tensor("x", [S, F], f32, kind="ExternalInput").ap()
    # w4 comes pre-swizzled from host: w4[p, 8*f + m] = W4[128*f + p, m]
    w4_d = nc.dram_tensor("w4", [128, 256], f32, kind="ExternalInput").ap()
    e4_d = nc.dram_tensor("e4", [R, F], f32, kind="ExternalInput").ap()
    id_d = nc.dram_tensor("ident", [128, 128], f32, kind="ExternalInput").ap()
    y_d = nc.dram_tensor("y", [S, F], f32, kind="ExternalOutput").ap()

    with tile.TileContext(nc) as tc, ExitStack() as ctx:
        const = ctx.enter_context(tc.tile_pool(name="const", bufs=1))
        xin = ctx.enter_context(tc.tile_pool(name="xin", bufs=3))
        xtp = ctx.enter_context(tc.tile_pool(name="xt", bufs=1))
        t4p = ctx.enter_context(tc.tile_pool(name="t4", bufs=1))
        outp = ctx.enter_context(tc.tile_pool(name="out", bufs=2))
        ps_tr = ctx.enter_context(
            tc.tile_pool(name="ps_tr", bufs=2, space=bass.MemorySpace.PSUM))
        ps_t4 = ctx.enter_context(
            tc.tile_pool(name="ps_t4", bufs=2, space=bass.MemorySpace.PSUM))
        ps_out = ctx.enter_context(
            tc.tile_pool(name="ps_out", bufs=4, space=bass.MemorySpace.PSUM))

        w4f = const.tile([128, 256], f32, tag="w4f")
        e4f = const.tile([R, F], f32, tag="e4f")
        ident = const.tile([128, 128], f32, tag="ident")
        nc.sync.dma_start(w4f[:], w4_d)
        nc.sync.dma_start(e4f[:], e4_d)
        nc.sync.dma_start(ident[:], id_d)
        # walrus requires f32r-matmul operands to be *produced* rounded to
        # f32r — bounce the factors through an ACT copy that rounds.
        w4 = const.tile([128, 256], f32r, tag="w4")
        e4 = const.tile([R, F], f32r, tag="e4")
        nc.scalar.copy(w4[:], w4f[:])
        nc.scalar.copy(e4[:], e4f[:])

        # Two s-halves (256 seq positions each) pipeline input vs output phases.
        # xt_h[h][p, 256*f + s] = X[256*h + s, 128*f + p]
        xt_h = [xtp.tile([128, 32 * 256], f32r, tag=f"xt{h}", name=f"xt{h}")
                for h in range(2)]
        t4_h = [t4p.tile([R, 256], f32r, tag=f"t4{h}", name=f"t4{h}")
                for h in range(2)]

        def load_and_transpose(c):
            xc = xin.tile([128, F], f32, tag="xc")
            nc.sync.dma_start(xc[:], x_d[c * 128:(c + 1) * 128, :])
            h, o = c // 2, (c % 2) * 128
            dst_all = xt_h[h][:].rearrange("p (f s) -> p f s", s=256)
            for fg in range(8):          # groups of 4 feat-chunks share a bank
                pt = ps_tr.tile([128, 512], f32, tag="pt")
                for j in range(4):
                    f = fg * 4 + j
                    nc.tensor.transpose(
                        pt[:, j * 128:(j + 1) * 128],
                        xc[:, f * 128:(f + 1) * 128],
                        ident[:],
                    )
                src = pt[:].rearrange("p (j s) -> p j s", s=128)
                dst = dst_all[:, fg * 4:(fg + 1) * 4, o:o + 128]
                nc.scalar.copy(dst, src)

        def mm1(h):
            acc = ps_t4.tile([R, 256], f32, tag="pacc")
            xt = xt_h[h]
            for f in range(32):
                nc.tensor.matmul(
                    acc[:],
                    w4[:, f * 8:(f + 1) * 8],
                    xt[:, f * 256:(f + 1) * 256],
                    start=(f == 0),
                    stop=(f == 31),
                )
            nc.scalar.copy(t4_h[h][:], acc[:])

        def mm2_store(c):
            h, o = c // 2, (c % 2) * 128
            orow = outp.tile([128, F], f32, tag="orow")
            for n in range(8):
                po = ps_out.tile([128, 512], f32, tag="po")
                nc.tensor.matmul(
                    po[:],
                    t4_h[h][:, o:o + 128],
                    e4[:, n * 512:(n + 1) * 512],
                    start=True,
                    stop=True,
                )
                nc.vector.tensor_copy(orow[:, n * 512:(n + 1) * 512], po[:])
            nc.sync.dma_start(y_d[c * 128:(c + 1) * 128, :], orow[:])

        load_and_transpose(0)
        load_and_transpose(1)
        mm1(0)
        mm2_store(0)
        mm2_store(1)
        load_and_transpose(2)
        load_and_transpose(3)
        mm1(1)
        mm2_store(2)
        mm2_store(3)

    nc.compile()
    return nc


def _get_program():
    global _PROGRAM
    if _PROGRAM is None:
        _PROGRAM = _build_program()
    return _PROGRAM


def _make_in_maps(X, gates, core_first, cores_mid, core_last):
    W4, E4 = _merge_factors(gates, core_first, cores_mid, core_last)
    ident = np.eye(128, dtype=np.float32)
    X = np.ascontiguousarray(np.asarray(X, np.float32))
    in_maps = []
    for b in range(B):
        w4sb = np.ascontiguousarray(
            W4[b].reshape(32, 128, R).transpose(1, 0, 2).reshape(128, 256))
        in_maps.append({
            "x": X[b],
            "w4": w4sb,
            "e4": np.ascontiguousarray(E4[b]),
            "ident": ident,
        })
    return in_maps


def kernel(X, gates, core_first, cores_mid, core_last):
    global LAST_RESULTS
    from concourse.bass_utils import run_bass_kernel_spmd

    nc = _get_program()
    in_maps = _make_in_maps(X, gates, core_first, cores_mid, core_last)
    res = run_bass_kernel_spmd(nc, in_maps, list(range(N_CORES)), trace=TRACE)
    LAST_RESULTS = res
    Y = np.stack([res.results[b]["y"] for b in range(B)], axis=0)
    return Y.astype(np.float32, copy=False)


# revision 7
# speedup vs baseline: 1.1058x; 1.1058x over previous
"""Trainium2 Bass kernel: gate-merged tensor-train (TT) MoE layer.

Reference math (per batch element b):
    merge each TT core over experts with gates[b]  ->  C_i  (tiny, <=512 floats)
    then an 8-step TT contraction maps X[b] : [512, 4096] -> Y[b] : [512, 4096].

Because every boundary rank is 8, the whole chain collapses exactly to a
rank-8 factorization:
    Y_b = X_b @ W4_b @ E4_b
with W4_b = C0*C1*C2*C3 reshaped to [4096, 8] and E4_b = C4*C5*C6*C7
reshaped to [8, 4096].  The factor merge is ~0.5 MFLOP of 8x8-sized einsums
(done on host at float64); all heavy lifting (16 MiB of streaming, the
4096-wide contractions) runs on the NeuronCores.

Sharding: data-parallel over batch B=8 across the 8 cores (one batch
element per core); the merged factors are per-core constants.

On-device dataflow per core:
    X  --DMA-->  SBUF (4 s-chunks of [128, 4096])
    PE transpose (fp32, identity matmul) -> PSUM -> ACT copy -> XT (feat-major)
    mm1 (f32r):  t4[8, s] += W4_chunk^T @ XT_chunk  (32 K=128 chunks, PSUM acc)
    mm2 (f32r):  Y[s-chunk, n-chunk] = t4_chunk^T @ E4  -> PSUM -> DVE -> SBUF
    DMA out, row-contiguous.
"""

import numpy as np

B = 8
S = 512
F = 4096
R = 8
N_CORES = 8

_PROGRAM = None
LAST_RESULTS = None  # BassKernelResults of the most recent run (for profiling)
TRACE = False        # set True (e.g. from test.py) to capture an NTFF profile


def _merge_factors(gates, core_first, cores_mid, core_last):
    """Host-side merge of the tiny TT cores -> per-batch rank-8 factors.

    Returns (W4, E4): [B, 4096, 8] and [B, 8, 4096] float32.
    """
    g = np.asarray(gates, np.float64)
    cf = np.asarray(core_first, np.float64)   # [E, 1, 8, 8]
    cm = np.asarray(cores_mid, np.float64)    # [6, E, 8, 8, 8]
    cl = np.asarray(core_last, np.float64)    # [E, 8, 8, 1]

    W4 = np.empty((B, F, R), np.float32)
    E4 = np.empty((B, R, F), np.float32)
    for b in range(B):
        C = [np.einsum('e,ermp->rmp', g[b], cf)] + \
            [np.einsum('e,ermp->rmp', g[b], cm[i]) for i in range(6)] + \
            [np.einsum('e,ermp->rmp', g[b], cl)]
        C0 = C[0][0]                                  # [m0, p1]
        W = np.einsum('ab,bic->iac', C0, C[1])        # [m1, m0, p2]
        W = np.einsum('iac,cjd->jiad', W, C[2])       # [m2, m1, m0, p3]
        W = np.einsum('jiad,dke->kjiae', W, C[3])     # [m3, m2, m1, m0, p4]
        W4[b] = W.reshape(F, R)
        E = np.einsum('anb,bmc->anmc', C[4], C[5])    # [p4, n0, n1, p6]
        E = np.einsum('anmc,ckd->anmkd', E, C[6])     # [p4, n0, n1, n2, p7]
        E = np.einsum('anmkd,dl->anmkl', E, C[7][:, :, 0])
        E4[b] = E.reshape(R, F)
    return W4, E4


def _build_program():
    """Build + compile the per-core Bass/Tile program (identical on all cores)."""
    from contextlib import ExitStack
    import concourse.bass as bass
    import concourse.tile as tile
    from concourse import bacc, mybir

    f32 = mybir.dt.float32
    bf16 = mybir.dt.bfloat16

    nc = bacc.Bacc("TRN2", target_bir_lowering=False, debug=False)
    x_d = nc.dram_tensor("x", [S, F], f32, kind="ExternalInput").ap()
    # w4 comes pre-swizzled from host: w4[p, 8*f + m] = W4[128*f + p, m]
    w4_d = nc.dram_tensor("w4", [128, 256], bf16, kind="ExternalInput").ap()
    e4_d = nc.dram_tensor("e4", [R, F], bf16, kind="ExternalInput").ap()
    id_d = nc.dram_tensor("ident", [128, 128], bf16, kind="ExternalInput").ap()
    y_d = nc.dram_tensor("y", [S, F], f32, kind="ExternalOutput").ap()

    with tile.TileContext(nc) as tc, ExitStack() as ctx:
        const = ctx.enter_context(tc.tile_pool(name="const", bufs=1))
        xin = ctx.enter_context(tc.tile_pool(name="xin", bufs=1))
        xtp = ctx.enter_context(tc.tile_pool(name="xt", bufs=1))
        t4p = ctx.enter_context(tc.tile_pool(name="t4", bufs=1))
        outp = ctx.enter_context(tc.tile_pool(name="out", bufs=2))
        ps_tr = ctx.enter_context(
            tc.tile_pool(name="ps_tr", bufs=2, space=bass.MemorySpace.PSUM))
        ps_t4 = ctx.enter_context(
            tc.tile_pool(name="ps_t4", bufs=1, space=bass.MemorySpace.PSUM))
        ps_out = ctx.enter_context(
            tc.tile_pool(name="ps_out", bufs=3, space=bass.MemorySpace.PSUM))

        # Kick all X loads first: SWDGE (gpsimd) DMAs cast f32 -> bf16
        # inline; consts ride the HWDGE (sync) ring so nothing head-blocks.
        # Each s-chunk is split into two feature-half tiles so transposes
        # start after the first half lands.
        xc = {}
        for c in range(4):
            for a in range(2):
                t = xin.tile([128, F // 2], bf16, tag=f"xc{c}{a}",
                             name=f"xc{c}{a}")
                nc.gpsimd.dma_start(
                    t[:], x_d[c * 128:(c + 1) * 128,
                              a * (F // 2):(a + 1) * (F // 2)])
                xc[c, a] = t

        w4 = const.tile([128, 256], bf16, tag="w4")
        e4 = const.tile([R, F], bf16, tag="e4")
        ident = const.tile([128, 128], bf16, tag="ident")
        nc.sync.dma_start(w4[:], w4_d)
        nc.sync.dma_start(e4[:], e4_d)
        nc.sync.dma_start(ident[:], id_d)

        # Two s-halves (256 seq positions each) pipeline input vs output phases.
        # xt_h[h][p, 256*f + s] = X[256*h + s, 128*f + p]
        xt_h = [xtp.tile([128, 32 * 256], bf16, tag=f"xt{h}", name=f"xt{h}")
                for h in range(2)]
        t4_h = [t4p.tile([R, 256], bf16, tag=f"t4{h}", name=f"t4{h}")
                for h in range(2)]

        def transpose_chunk(c):
            # Transpose [128 s, 4096 f] via regular bf16 matmuls against the
            # identity: out = xc_slice^T @ I.  Exact (multiplies by 1), and
            # unlike transpose-mode these warm the PE clock gate (HAM).
            h, o = c // 2, (c % 2) * 128
            dst_all = xt_h[h][:].rearrange("p (f s) -> p f s", s=256)
            for fg in range(4):          # 8 feat-chunks share a 2-bank tile
                pt = ps_tr.tile([128, 1024], f32, tag="pt")
                for j in range(8):
                    f = fg * 8 + j
                    src = xc[c, f // 16]
                    fo = f % 16
                    nc.tensor.matmul(
                        pt[:, j * 128:(j + 1) * 128],
                        src[:, fo * 128:(fo + 1) * 128],
                        ident[:],
                        start=True, stop=True,
                    )
                src_ap = pt[:].rearrange("p (j s) -> p j s", s=128)
                dst = dst_all[:, fg * 8:(fg + 1) * 8, o:o + 128]
                nc.scalar.copy(dst, src_ap)

        def mm1(h):
            acc = ps_t4.tile([R, 256], f32, tag="pacc")
            xt = xt_h[h]
            for f in range(32):
                nc.tensor.matmul(
                    acc[:],
                    w4[:, f * 8:(f + 1) * 8],
                    xt[:, f * 256:(f + 1) * 256],
                    start=(f == 0),
                    stop=(f == 31),
                )
            nc.scalar.copy(t4_h[h][:], acc[:])

        def mm2_store(c):
            h, o = c // 2, (c % 2) * 128
            orow = outp.tile([128, F], f32, tag="orow")
            for n in range(8):
                po = ps_out.tile([128, 512], f32, tag="po")
                nc.tensor.matmul(
                    po[:],
                    t4_h[h][:, o:o + 128],
                    e4[:, n * 512:(n + 1) * 512],
                    start=True,
                    stop=True,
                )
                nc.vector.tensor_copy(orow[:, n * 512:(n + 1) * 512], po[:])
            nc.sync.dma_start(y_d[c * 128:(c + 1) * 128, :], orow[:])

        transpose_chunk(0)
        transpose_chunk(1)
        mm1(0)
        mm2_store(0)
        mm2_store(1)
        transpose_chunk(2)
        transpose_chunk(3)
        mm1(1)
        mm2_store(2)
        mm2_store(3)

    nc.compile()
    return nc


def _get_program():
    global _PROGRAM
    if _PROGRAM is None:
        _PROGRAM = _build_program()
    return _PROGRAM


def _make_in_maps(X, gates, core_first, cores_mid, core_last):
    import ml_dtypes
    bf16 = ml_dtypes.bfloat16
    W4, E4 = _merge_factors(gates, core_first, cores_mid, core_last)
    ident = np.eye(128, dtype=bf16)
    X = np.ascontiguousarray(np.asarray(X, np.float32))
    in_maps = []
    for b in range(B):
        w4sb = np.ascontiguousarray(
            W4[b].reshape(32, 128, R).transpose(1, 0, 2).reshape(128, 256)
            .astype(bf16))
        in_maps.append({
            "x": X[b],
            "w4": w4sb,
            "e4": np.ascontiguousarray(E4[b].astype(bf16)),
            "ident": ident,
        })
    return in_maps


def kernel(X, gates, core_first, cores_mid, core_last):
    global LAST_RESULTS
    from concourse.bass_utils import run_bass_kernel_spmd

    nc = _get_program()
    in_maps = _make_in_maps(X, gates, core_first, cores_mid, core_last)
    res = run_bass_kernel_spmd(nc, in_maps, list(range(N_CORES)), trace=TRACE)
    LAST_RESULTS = res
    Y = np.stack([res.results[b]["y"] for b in range(B)], axis=0)
    return Y.astype(np.float32, copy=False)


# revision 10
# speedup vs baseline: 1.1725x; 1.0604x over previous
"""Trainium2 Bass kernel: gate-merged tensor-train (TT) MoE layer.

Reference math (per batch element b):
    merge each TT core over experts with gates[b]  ->  C_i  (tiny, <=512 floats)
    then an 8-step TT contraction maps X[b] : [512, 4096] -> Y[b] : [512, 4096].

Because every boundary rank is 8, the whole chain collapses exactly to a
rank-8 factorization:
    Y_b = X_b @ W4_b @ E4_b
with W4_b = C0*C1*C2*C3 reshaped to [4096, 8] and E4_b = C4*C5*C6*C7
reshaped to [8, 4096].  The factor merge is ~0.5 MFLOP of 8x8-sized einsums
(done on host at float64); all heavy lifting (16 MiB of streaming, the
4096-wide contractions) runs on the NeuronCores.

Sharding: data-parallel over batch B=8 across the 8 cores (one batch
element per core); the merged factors are per-core constants.

On-device dataflow per core:
    X  --DMA-->  SBUF (4 s-chunks of [128, 4096])
    PE transpose (fp32, identity matmul) -> PSUM -> ACT copy -> XT (feat-major)
    mm1 (f32r):  t4[8, s] += W4_chunk^T @ XT_chunk  (32 K=128 chunks, PSUM acc)
    mm2 (f32r):  Y[s-chunk, n-chunk] = t4_chunk^T @ E4  -> PSUM -> DVE -> SBUF
    DMA out, row-contiguous.
"""

import numpy as np

B = 8
S = 512
F = 4096
R = 8
N_CORES = 8

_PROGRAM = None
LAST_RESULTS = None  # BassKernelResults of the most recent run (for profiling)
TRACE = False        # set True (e.g. from test.py) to capture an NTFF profile


def _merge_factors(gates, core_first, cores_mid, core_last):
    """Host-side merge of the tiny TT cores -> per-batch rank-8 factors.

    Returns (W4, E4): [B, 4096, 8] and [B, 8, 4096] float32.
    """
    g = np.asarray(gates, np.float64)
    cf = np.asarray(core_first, np.float64)   # [E, 1, 8, 8]
    cm = np.asarray(cores_mid, np.float64)    # [6, E, 8, 8, 8]
    cl = np.asarray(core_last, np.float64)    # [E, 8, 8, 1]

    W4 = np.empty((B, F, R), np.float32)
    E4 = np.empty((B, R, F), np.float32)
    for b in range(B):
        C = [np.einsum('e,ermp->rmp', g[b], cf)] + \
            [np.einsum('e,ermp->rmp', g[b], cm[i]) for i in range(6)] + \
            [np.einsum('e,ermp->rmp', g[b], cl)]
        C0 = C[0][0]                                  # [m0, p1]
        W = np.einsum('ab,bic->iac', C0, C[1])        # [m1, m0, p2]
        W = np.einsum('iac,cjd->jiad', W, C[2])       # [m2, m1, m0, p3]
        W = np.einsum('jiad,dke->kjiae', W, C[3])     # [m3, m2, m1, m0, p4]
        W4[b] = W.reshape(F, R)
        E = np.einsum('anb,bmc->anmc', C[4], C[5])    # [p4, n0, n1, p6]
        E = np.einsum('anmc,ckd->anmkd', E, C[6])     # [p4, n0, n1, n2, p7]
        E = np.einsum('anmkd,dl->anmkl', E, C[7][:, :, 0])
        E4[b] = E.reshape(R, F)
    return W4, E4


def _build_program():
    """Build + compile the per-core Bass/Tile program (identical on all cores)."""
    from contextlib import ExitStack
    import concourse.bass as bass
    import concourse.tile as tile
    from concourse import bacc, mybir

    f32 = mybir.dt.float32
    bf16 = mybir.dt.bfloat16

    nc = bacc.Bacc("TRN2", target_bir_lowering=False, debug=False)
    x_d = nc.dram_tensor("x", [S, F], f32, kind="ExternalInput").ap()
    # w4 comes pre-swizzled from host: w4[p, 8*f + m] = W4[128*f + p, m]
    w4_d = nc.dram_tensor("w4", [128, 256], bf16, kind="ExternalInput").ap()
    e4_d = nc.dram_tensor("e4", [R, F], bf16, kind="ExternalInput").ap()
    id_d = nc.dram_tensor("ident", [128, 128], bf16, kind="ExternalInput").ap()
    y_d = nc.dram_tensor("y", [S, F], f32, kind="ExternalOutput").ap()

    with tile.TileContext(nc) as tc, ExitStack() as ctx:
        const = ctx.enter_context(tc.tile_pool(name="const", bufs=1))
        xin = ctx.enter_context(tc.tile_pool(name="xin", bufs=1))
        xtp = ctx.enter_context(tc.tile_pool(name="xt", bufs=1))
        t4p = ctx.enter_context(tc.tile_pool(name="t4", bufs=1))
        outp = ctx.enter_context(tc.tile_pool(name="out", bufs=2))
        ps_tr = ctx.enter_context(
            tc.tile_pool(name="ps_tr", bufs=3, space=bass.MemorySpace.PSUM))
        ps_t4 = ctx.enter_context(
            tc.tile_pool(name="ps_t4", bufs=1, space=bass.MemorySpace.PSUM))
        ps_out = ctx.enter_context(
            tc.tile_pool(name="ps_out", bufs=3, space=bass.MemorySpace.PSUM))

        # Kick all X loads first: SWDGE (gpsimd) DMAs cast f32 -> bf16
        # inline; consts ride the HWDGE (sync) ring so nothing head-blocks.
        # Each s-chunk is split into two feature-half tiles so transposes
        # start after the first half lands.
        xc = {}
        for c in range(4):
            for a in range(2):
                t = xin.tile([128, F // 2], bf16, tag=f"xc{c}{a}",
                             name=f"xc{c}{a}")
                nc.gpsimd.dma_start(
                    t[:], x_d[c * 128:(c + 1) * 128,
                              a * (F // 2):(a + 1) * (F // 2)])
                xc[c, a] = t

        w4 = const.tile([128, 256], bf16, tag="w4")
        e4 = const.tile([R, F], bf16, tag="e4")
        ident = const.tile([128, 128], bf16, tag="ident")
        nc.sync.dma_start(w4[:], w4_d)
        nc.sync.dma_start(e4[:], e4_d)
        nc.sync.dma_start(ident[:], id_d)

        # Two s-halves (256 seq positions each) pipeline input vs output phases.
        # xt_h[h][p, 256*f + s] = X[256*h + s, 128*f + p]
        xt_h = [xtp.tile([128, 32 * 256], bf16, tag=f"xt{h}", name=f"xt{h}")
                for h in range(2)]
        t4_h = [t4p.tile([R, 256], bf16, tag=f"t4{h}", name=f"t4{h}")
                for h in range(2)]

        def transpose_chunk(c):
            # Transpose [128 s, 4096 f] via regular bf16 matmuls against the
            # identity: out = xc_slice^T @ I.  Exact (multiplies by 1), and
            # unlike transpose-mode these warm the PE clock gate (HAM).
            # Drains alternate ACT/DVE so the PSUM drain rate keeps up with
            # the PE fill rate and the PE never micro-idles.
            h, o = c // 2, (c % 2) * 128
            dst_all = xt_h[h][:].rearrange("p (f s) -> p f s", s=256)
            for fg in range(8):          # 4 feat-chunks share a 1-bank tile
                pt = ps_tr.tile([128, 512], f32, tag="pt")
                for j in range(4):
                    f = fg * 4 + j
                    src = xc[c, f // 16]
                    fo = f % 16
                    nc.tensor.matmul(
                        pt[:, j * 128:(j + 1) * 128],
                        src[:, fo * 128:(fo + 1) * 128],
                        ident[:],
                        start=True, stop=True,
                    )
                src_ap = pt[:].rearrange("p (j s) -> p j s", s=128)
                dst = dst_all[:, fg * 4:(fg + 1) * 4, o:o + 128]
                if fg % 2 == 0:
                    nc.scalar.copy(dst, src_ap)
                else:
                    nc.vector.tensor_copy(dst, src_ap)

        def mm1(h):
            acc = ps_t4.tile([R, 256], f32, tag="pacc")
            xt = xt_h[h]
            for f in range(32):
                nc.tensor.matmul(
                    acc[:],
                    w4[:, f * 8:(f + 1) * 8],
                    xt[:, f * 256:(f + 1) * 256],
                    start=(f == 0),
                    stop=(f == 31),
                )
            nc.scalar.copy(t4_h[h][:], acc[:])

        def mm2_store(c):
            # Output rows in two feature-half tiles so each 1 MiB store can
            # start after 4 drains; drains alternate DVE/ACT.
            h, o = c // 2, (c % 2) * 128
            for a in range(2):
                orow = outp.tile([128, F // 2], f32, tag=f"orow{a}",
                                 name=f"orow{a}")
                for k in range(4):
                    n = a * 4 + k
                    po = ps_out.tile([128, 512], f32, tag="po")
                    nc.tensor.matmul(
                        po[:],
                        t4_h[h][:, o:o + 128],
                        e4[:, n * 512:(n + 1) * 512],
                        start=True,
                        stop=True,
                    )
                    dst = orow[:, k * 512:(k + 1) * 512]
                    if n % 2 == 0:
                        nc.vector.tensor_copy(dst, po[:])
                    else:
                        nc.scalar.copy(dst, po[:])
                nc.sync.dma_start(
                    y_d[c * 128:(c + 1) * 128,
                        a * (F // 2):(a + 1) * (F // 2)], orow[:])

        transpose_chunk(0)
        transpose_chunk(1)
        mm1(0)
        mm2_store(0)
        mm2_store(1)
        transpose_chunk(2)
        transpose_chunk(3)
        mm1(1)
        mm2_store(2)
        mm2_store(3)

    nc.compile()
    return nc


def _get_program():
    global _PROGRAM
    if _PROGRAM is None:
        _PROGRAM = _build_program()
    return _PROGRAM


def _make_in_maps(X, gates, core_first, cores_mid, core_last):
    import ml_dtypes
    bf16 = ml_dtypes.bfloat16
    W4, E4 = _merge_factors(gates, core_first, cores_mid, core_last)
    ident = np.eye(128, dtype=bf16)
    X = np.ascontiguousarray(np.asarray(X, np.float32))
    in_maps = []
    for b in range(B):
        w4sb = np.ascontiguousarray(
            W4[b].reshape(32, 128, R).transpose(1, 0, 2).reshape(128, 256)
            .astype(bf16))
        in_maps.append({
            "x": X[b],
            "w4": w4sb,
            "e4": np.ascontiguousarray(E4[b].astype(bf16)),
            "ident": ident,
        })
    return in_maps


def kernel(X, gates, core_first, cores_mid, core_last):
    global LAST_RESULTS
    from concourse.bass_utils import run_bass_kernel_spmd

    nc = _get_program()
    in_maps = _make_in_maps(X, gates, core_first, cores_mid, core_last)
    res = run_bass_kernel_spmd(nc, in_maps, list(range(N_CORES)), trace=TRACE)
    LAST_RESULTS = res
    Y = np.stack([res.results[b]["y"] for b in range(B)], axis=0)
    return Y.astype(np.float32, copy=False)
